# revision 7
# baseline (speedup 1.0000x reference)
"""Trainium2 Bass kernel: gated linear recurrence encoder.

Math (per example):
    z = tanh(x @ Wz.T + bz)        # [T, D]
    o = tanh(x @ Wo.T + bo)        # [T, D]
    c_t = g_t * c_{t-1} + (1 - g_t) * z_t   (c_{-1} = 0)
    h_t = o_t * c_t

Sharding: batch B=64 split across 8 cores (8 examples per core); weights
replicated.  Device-side layout is feature-major [D, T] per example (host
pre-transposes), so:
  - matmuls produce z^T/o^T directly ([e, t], contraction over d on the
    partition axis),
  - the recurrence runs as a single hardware tensor_tensor_scan along the
    free (time) axis per 100-feature chunk,
  - all DMA is fully contiguous; the host untransposes the output.
"""

import numpy as np

B, T, D = 64, 2048, 300
N_CORES = 8
BL = B // N_CORES      # examples per core
DC = 100               # feature-chunk size (3 chunks of 100 = D)
NCH = D // DC          # 3
NT = 512               # matmul moving-dim tile (max for 4-byte dtypes)
NNT = T // NT          # 4

_CACHE = {}
PROFILE = False        # set True (e.g. from test.py) to capture an NTFF trace
LAST_RESULTS = None    # BassKernelResults of the most recent run


def _build_nc(bl=BL):
    import concourse.bass as bass
    import concourse.bacc as bacc
    import concourse.mybir as mybir
    from concourse import tile

    f32 = mybir.dt.float32
    f32r = mybir.dt.float32r
    Alu = mybir.AluOpType
    ActF = mybir.ActivationFunctionType

    nc = bacc.Bacc()
    xT = nc.dram_tensor("xT", [bl, D, T], f32r, kind="ExternalInput")
    gT = nc.dram_tensor("gT", [bl, D, T], f32, kind="ExternalInput")
    WzT = nc.dram_tensor("WzT", [D, D], f32r, kind="ExternalInput")
    WoT = nc.dram_tensor("WoT", [D, D], f32r, kind="ExternalInput")
    bz = nc.dram_tensor("bz", [D, 1], f32, kind="ExternalInput")
    bo = nc.dram_tensor("bo", [D, 1], f32, kind="ExternalInput")
    hT = nc.dram_tensor("hT", [bl, D, T], f32, kind="ExternalOutput")

    with tile.TileContext(nc) as tc:
        with (
            tc.tile_pool(name="wpool", bufs=1) as wpool,
            tc.tile_pool(name="xpool", bufs=2) as xpool,
            tc.tile_pool(name="gpool", bufs=2) as gpool,
            tc.tile_pool(name="zpool", bufs=2) as zpool,
            tc.tile_pool(name="vpool", bufs=2) as vpool,
            tc.tile_pool(name="psum", bufs=2, space="PSUM") as psum,
        ):
            # Weights: lhsT chunks [k=d_in (partitions), e_out (free)]
            wz_t, wo_t, bz_t, bo_t = [], [], [], []
            for k in range(NCH):
                wzk = wpool.tile([DC, D], f32r, tag=f"wz{k}")
                nc.gpsimd.dma_start(wzk[:], WzT[k * DC:(k + 1) * DC, :])
                wz_t.append(wzk)
                wok = wpool.tile([DC, D], f32r, tag=f"wo{k}")
                nc.gpsimd.dma_start(wok[:], WoT[k * DC:(k + 1) * DC, :])
                wo_t.append(wok)
            for j in range(NCH):
                bzj = wpool.tile([DC, 1], f32, tag=f"bz{j}")
                nc.gpsimd.dma_start(bzj[:], bz[j * DC:(j + 1) * DC, :])
                bz_t.append(bzj)
                boj = wpool.tile([DC, 1], f32, tag=f"bo{j}")
                nc.gpsimd.dma_start(boj[:], bo[j * DC:(j + 1) * DC, :])
                bo_t.append(boj)

            for b in range(bl):
                # x^T for this example: all 3 k-chunks (each [100, T])
                xk = []
                for k in range(NCH):
                    xkt = xpool.tile([DC, T], f32r, tag=f"x{k}")
                    nc.sync.dma_start(xkt[:], xT[b, k * DC:(k + 1) * DC, :])
                    xk.append(xkt)
                for j in range(NCH):
                    gt = gpool.tile([DC, T], f32, tag="g")
                    nc.sync.dma_start(gt[:], gT[b, j * DC:(j + 1) * DC, :])
                    zt = zpool.tile([DC, T], f32, tag="z")
                    ot = zpool.tile([DC, T], f32, tag="o")
                    for n in range(NNT):
                        ns = slice(n * NT, (n + 1) * NT)
                        pz = psum.tile([DC, NT], f32, tag="pz")
                        for k in range(NCH):
                            nc.tensor.matmul(
                                pz[:],
                                wz_t[k][:, j * DC:(j + 1) * DC],
                                xk[k][:, ns],
                                start=(k == 0),
                                stop=(k == NCH - 1),
                            )
                        nc.scalar.activation(
                            zt[:, ns], pz[:], ActF.Tanh, bias=bz_t[j][:]
                        )
                        po = psum.tile([DC, NT], f32, tag="po")
                        for k in range(NCH):
                            nc.tensor.matmul(
                                po[:],
                                wo_t[k][:, j * DC:(j + 1) * DC],
                                xk[k][:, ns],
                                start=(k == 0),
                                stop=(k == NCH - 1),
                            )
                        nc.scalar.activation(
                            ot[:, ns], po[:], ActF.Tanh, bias=bo_t[j][:]
                        )
                    # d1 = (g - 1) * z ; scan: c = g*c_prev - d1 = g*c_prev + (1-g)*z
                    d1 = vpool.tile([DC, T], f32, tag="d1")
                    nc.vector.scalar_tensor_tensor(
                        d1[:], gt[:], 1.0, zt[:], op0=Alu.subtract, op1=Alu.mult
                    )
                    ct = vpool.tile([DC, T], f32, tag="c")
                    nc.vector.tensor_tensor_scan(
                        ct[:], gt[:], d1[:], 0.0, op0=Alu.mult, op1=Alu.subtract
                    )
                    ht = vpool.tile([DC, T], f32, tag="h")
                    nc.vector.tensor_mul(ht[:], ot[:], ct[:])
                    nc.sync.dma_start(hT[b, j * DC:(j + 1) * DC, :], ht[:])
    nc.compile()
    return nc


def _get_nc():
    if "nc" not in _CACHE:
        _CACHE["nc"] = _build_nc()
    return _CACHE["nc"]


def _make_in_maps(gate_encoding, inputs_encoding, Wz, bz, Wo, bo):
    gate_encoding = np.asarray(gate_encoding, dtype=np.float32)
    inputs_encoding = np.asarray(inputs_encoding, dtype=np.float32)
    WzT = np.ascontiguousarray(np.asarray(Wz, dtype=np.float32).T)
    WoT = np.ascontiguousarray(np.asarray(Wo, dtype=np.float32).T)
    bz2 = np.ascontiguousarray(np.asarray(bz, dtype=np.float32).reshape(D, 1))
    bo2 = np.ascontiguousarray(np.asarray(bo, dtype=np.float32).reshape(D, 1))

    in_maps = []
    for c in range(N_CORES):
        sl = slice(c * BL, (c + 1) * BL)
        in_maps.append({
            "xT": np.ascontiguousarray(inputs_encoding[sl].transpose(0, 2, 1)),
            "gT": np.ascontiguousarray(gate_encoding[sl].transpose(0, 2, 1)),
            "WzT": WzT,
            "WoT": WoT,
            "bz": bz2,
            "bo": bo2,
        })
    return in_maps


def kernel(gate_encoding, inputs_encoding, Wz, bz, Wo, bo):
    from concourse.bass_utils import run_bass_kernel_spmd

    nc = _get_nc()
    in_maps = _make_in_maps(gate_encoding, inputs_encoding, Wz, bz, Wo, bo)
    res = run_bass_kernel_spmd(nc, in_maps, list(range(N_CORES)), trace=PROFILE)
    global LAST_RESULTS
    LAST_RESULTS = res

    hT_full = np.concatenate([r["hT"] for r in res.results], axis=0)  # [B, D, T]
    return np.ascontiguousarray(hT_full.transpose(0, 2, 1))  # [B, T, D]
